# revision 9
# baseline (speedup 1.0000x reference)
"""Multi-head latent attention (MLA) prefill kernel for 8 Trainium2 NeuronCores.

Sharding strategy (tensor-parallel over heads + token-parallel projections):
  Phase A (token-parallel): each core computes the compressed latents c_q/c_kv
    (feature-major) for its 512 of the 4096 tokens, then an AllGather shares
    the full latents with every core.
  Phase B (head-parallel): each core computes k_r (RoPE branch), the
    up-projections, and causal attention for its 2 of the 16 heads (both
    batches), producing attention output O_T [256 dims, 4096 tokens].
  Phase C: an AllToAll token-shards O_T so each core applies the full
    out-projection to its 512 tokens. The host concatenates the shards.

All matmuls run as float32r (FP22 truncation, full PE rate at N>=512).
Everything on-chip is kept feature-major so every matmul has N=512 moving dim.
Softmax is computed k-major without max subtraction (scores are ~N(0,1) after
scaling, so exp cannot overflow); denominators come from a ones-vector matmul
and are broadcast across partitions with a rank-1 outer-product matmul.
"""

import sys
import types

sys.path.insert(0, "/opt/trn_rl_repo")

import numpy as np

from concourse import bacc, bass, mybir, tile
from concourse import bass_utils

F32 = mybir.dt.float32
F32R = mybir.dt.float32r
AF = mybir.ActivationFunctionType

E = 2048
H = 16
HD = 128
CKV = 512
CQ = 1536
RD = 64
SCALE = 1.0 / np.sqrt(HD + RD)
B = 2
S = 2048
T = B * S            # 4096 tokens
NC = 8               # cores
TPC = T // NC        # 512 tokens per core
HPC = H // NC        # 2 heads per core
NB = T // 512        # 8 token blocks of 512
NBB = S // 512       # 4 token blocks per batch
ET = E // 128        # 16 e-tiles
CQT = CQ // 128      # 12 c_q tiles
CKVT = CKV // 128    # 4 c_kv tiles


def r(ap):
    """Reinterpret an fp32 AP as float32r for full-rate PE matmuls."""
    return ap.bitcast(F32R)


def build_program():
    nc = bacc.Bacc("TRN2", target_bir_lowering=False, debug=False, num_devices=NC)

    # ---- I/O ----
    xt_loc = nc.dram_tensor("xt_loc", [E, TPC], F32R, kind="ExternalInput")
    xt_full = nc.dram_tensor("xt_full", [E, T], F32R, kind="ExternalInput")
    wdq_t = nc.dram_tensor("wdq_t", [E, CQ], F32R, kind="ExternalInput")
    wdkv_t = nc.dram_tensor("wdkv_t", [E, CKV], F32R, kind="ExternalInput")
    wkr_t = nc.dram_tensor("wkr_t", [E, HPC * RD], F32R, kind="ExternalInput")
    wuq_t = nc.dram_tensor("wuq_t", [CQ, HPC * HD], F32R, kind="ExternalInput")
    wqr_t = nc.dram_tensor("wqr_t", [CQ, HPC * RD], F32R, kind="ExternalInput")
    wuk_t = nc.dram_tensor("wuk_t", [CKV, HPC * HD], F32R, kind="ExternalInput")
    wuv_t = nc.dram_tensor("wuv_t", [CKV, HPC * HD], F32R, kind="ExternalInput")
    wout_t = nc.dram_tensor("wout_t", [E, E], F32R, kind="ExternalInput")
    cos_t = nc.dram_tensor("cos_t", [128, T], F32, kind="ExternalInput")
    sin_t = nc.dram_tensor("sin_t", [128, T], F32, kind="ExternalInput")
    mask_t = nc.dram_tensor("mask_t", [128, 4 * 512], F32, kind="ExternalInput")
    ones_t = nc.dram_tensor("ones_t", [128, 128], F32R, kind="ExternalInput")
    ident = nc.dram_tensor("ident", [128, 128], F32, kind="ExternalInput")
    out_t = nc.dram_tensor("out_t", [E, TPC], F32, kind="ExternalOutput")

    # ---- internal DRAM (collective bounce buffers) ----
    ag_in = nc.dram_tensor("ag_in", [CQ + CKV, TPC], F32R)
    ag_out = nc.dram_tensor("ag_out", [NC * (CQ + CKV), TPC], F32R)
    a2a_in = nc.dram_tensor("a2a_in", [NC * HPC * HD, 512], F32R)
    a2a_out = nc.dram_tensor("a2a_out", [NC * HPC * HD, 512], F32R)

    rg = [list(range(NC))]

    with tile.TileContext(nc) as tc:
        # ================= Phase A: down-projections (own 512 tokens) ======
        with (
            tc.tile_pool(name="pa_x", bufs=1) as pa_x,
            tc.tile_pool(name="pa_w", bufs=3) as pa_w,
            tc.tile_pool(name="pa_s", bufs=3) as pa_s,
            tc.tile_pool(name="pa_ps", bufs=2, space="PSUM") as pa_ps,
        ):
            x_sb = pa_x.tile([128, ET * TPC], F32R)
            for e in range(ET):
                nc.sync.dma_start(
                    x_sb[:, e * TPC : (e + 1) * TPC],
                    xt_loc[e * 128 : (e + 1) * 128, :],
                )
            for w_dram, n_chunks, row_base in ((wdq_t, CQT, 0), (wdkv_t, CKVT, CQ)):
                for m in range(n_chunks):
                    w_sb = pa_w.tile([128, ET * 128], F32R, tag="wa", bufs=3)
                    for e in range(ET):
                        nc.sync.dma_start(
                            w_sb[:, e * 128 : (e + 1) * 128],
                            w_dram[e * 128 : (e + 1) * 128, m * 128 : (m + 1) * 128],
                        )
                    ps = pa_ps.tile([128, TPC], F32, tag="pa", bufs=2)
                    for e in range(ET):
                        nc.tensor.matmul(
                            ps[:],
                            r(w_sb[:, e * 128 : (e + 1) * 128]),
                            r(x_sb[:, e * TPC : (e + 1) * TPC]),
                            start=(e == 0),
                            stop=(e == ET - 1),
                        )
                    o_sb = pa_s.tile([128, TPC], F32R, tag="oa", bufs=3)
                    nc.vector.tensor_copy(o_sb[:], ps[:])
                    row = row_base + m * 128
                    nc.sync.dma_start(ag_in[row : row + 128, :], o_sb[:])

        nc.gpsimd.collective_compute(
            "AllGather",
            mybir.AluOpType.bypass,
            replica_groups=rg,
            ins=[ag_in.ap().opt()],
            outs=[ag_out.ap().opt()],
        )

        # ================= Phase B: heads (2 per core), both batches ========
        with (
            tc.tile_pool(name="pb_const", bufs=1) as pb_const,
            tc.tile_pool(name="pb_res", bufs=1) as pb_res,
            tc.tile_pool(name="pb_stream", bufs=3) as pb_stream,
            tc.tile_pool(name="pb_unit", bufs=1) as pb_unit,
            tc.tile_pool(name="pb_small", bufs=3) as pb_small,
            tc.tile_pool(name="ps_u", bufs=2, space="PSUM") as ps_u,
            tc.tile_pool(name="ps_s", bufs=2, space="PSUM") as ps_s,
            tc.tile_pool(name="ps_o", bufs=1, space="PSUM") as ps_o,
            tc.tile_pool(name="ps_misc", bufs=1, space="PSUM") as ps_misc,
        ):
            # constants
            id_sb = pb_const.tile([128, 128], F32)
            nc.sync.dma_start(id_sb[:], ident[:, :])
            mask_sb = pb_const.tile([128, 4 * 512], F32)
            nc.sync.dma_start(mask_sb[:], mask_t[:, :])
            ones_sb = pb_const.tile([128, 128], F32R)
            nc.sync.dma_start(ones_sb[:], ones_t[:, :])
            ones_k = ones_sb[:, 0:1]
            ones_m = ones_sb[0:1, :]
            # up-projection weights, resident all of phase B
            wuq_sb = pb_const.tile([128, CQT * HPC * HD], F32R)
            for c in range(CQT):
                nc.sync.dma_start(
                    wuq_sb[:, c * 256 : (c + 1) * 256],
                    wuq_t[c * 128 : (c + 1) * 128, :],
                )
            wqr_sb = pb_const.tile([128, CQT * HPC * RD], F32R)
            for c in range(CQT):
                nc.sync.dma_start(
                    wqr_sb[:, c * 128 : (c + 1) * 128],
                    wqr_t[c * 128 : (c + 1) * 128, :],
                )
            wuk_sb = pb_const.tile([128, CKVT * HPC * HD], F32R)
            wuv_sb = pb_const.tile([128, CKVT * HPC * HD], F32R)
            for c in range(CKVT):
                nc.sync.dma_start(
                    wuk_sb[:, c * 256 : (c + 1) * 256],
                    wuk_t[c * 128 : (c + 1) * 128, :],
                )
                nc.sync.dma_start(
                    wuv_sb[:, c * 256 : (c + 1) * 256],
                    wuv_t[c * 128 : (c + 1) * 128, :],
                )
            wkr_sb = pb_const.tile([128, ET * 128], F32R)
            for e in range(ET):
                nc.sync.dma_start(
                    wkr_sb[:, e * 128 : (e + 1) * 128],
                    wkr_t[e * 128 : (e + 1) * 128, :],
                )

            def rope(dst, src_ps, tb, nrows):
                """dst[:nrows, 512] = rope(src_ps[:nrows, 512]) for token block tb.

                Rows are 64-dim RoPE blocks (one per head); rotate-half pairs
                row d with d+32 inside each block. sin_t comes pre-signed.
                """
                cos_sb = pb_small.tile([128, 512], F32, tag="cos", bufs=1)
                sin_sb = pb_small.tile([128, 512], F32, tag="sin", bufs=1)
                nc.sync.dma_start(cos_sb[:nrows], cos_t[0:nrows, tb * 512 : (tb + 1) * 512])
                nc.sync.dma_start(sin_sb[:nrows], sin_t[0:nrows, tb * 512 : (tb + 1) * 512])
                sh = pb_small.tile([128, 512], F32, tag="sh", bufs=1)
                for blk in range(nrows // 64):
                    p0 = blk * 64
                    nc.vector.tensor_copy(sh[p0 : p0 + 32, :], src_ps[p0 + 32 : p0 + 64, :])
                    nc.vector.tensor_copy(sh[p0 + 32 : p0 + 64, :], src_ps[p0 : p0 + 32, :])
                t1 = pb_small.tile([128, 512], F32, tag="t1", bufs=1)
                nc.vector.tensor_mul(t1[:nrows], src_ps[:nrows], cos_sb[:nrows])
                nc.vector.tensor_mul(sh[:nrows], sh[:nrows], sin_sb[:nrows])
                nc.vector.tensor_add(dst, t1[:nrows], sh[:nrows])

            # ---- B0: k_r for this core's 2 heads, all 4096 tokens ----
            kr_sb = pb_res.tile([128, T], F32R)
            for tb in range(NB):
                ps = ps_u.tile([128, 512], F32, tag="u", bufs=2)
                for e in range(ET):
                    xf_sb = pb_stream.tile(
                        [128, 512], F32R, tag="xf", bufs=4, name=f"xf{tb}_{e}"
                    )
                    nc.sync.dma_start(
                        xf_sb[:],
                        xt_full[e * 128 : (e + 1) * 128, tb * 512 : (tb + 1) * 512],
                    )
                    nc.tensor.matmul(
                        ps[:],
                        r(wkr_sb[:, e * 128 : (e + 1) * 128]),
                        r(xf_sb[:]),
                        start=(e == 0),
                        stop=(e == ET - 1),
                    )
                rope(kr_sb[:, tb * 512 : (tb + 1) * 512], ps, tb, 128)

            # ---- B1+B2 per batch: up-projections then attention ----
            for b in range(B):
                qc_u = [
                    pb_unit.tile([128, S], F32R, tag=f"qc{h}", bufs=1, name=f"qc{h}_b{b}")
                    for h in range(HPC)
                ]
                kc_u = [
                    pb_unit.tile([128, S], F32R, tag=f"kc{h}", bufs=1, name=f"kc{h}_b{b}")
                    for h in range(HPC)
                ]
                vk_u = [
                    pb_unit.tile([128, S], F32R, tag=f"vk{h}", bufs=1, name=f"vk{h}_b{b}")
                    for h in range(HPC)
                ]
                qr_u = pb_unit.tile([128, S], F32R, tag="qr", bufs=1, name=f"qr_b{b}")

                for tbl in range(NBB):
                    tb = b * NBB + tbl
                    col = slice(tbl * 512, (tbl + 1) * 512)
                    # stream the gathered latents for this token block.  All
                    # 12 c_q tiles are live within one accumulation group, so
                    # the tag needs 12 slots (+1 for prefetch overlap).
                    cq_tiles = []
                    for c in range(CQT):
                        cq = pb_stream.tile([128, 512], F32R, tag="cq", bufs=13, name=f"cq{c}_{tb}")
                        row = tb * (CQ + CKV) + c * 128
                        nc.sync.dma_start(cq[:], ag_out[row : row + 128, :])
                        cq_tiles.append(cq)
                    ckv_tiles = []
                    for c in range(CKVT):
                        ckv = pb_stream.tile([128, 512], F32R, tag="ckv", bufs=4, name=f"ckv{c}_{tb}")
                        row = tb * (CQ + CKV) + CQ + c * 128
                        nc.sync.dma_start(ckv[:], ag_out[row : row + 128, :])
                        ckv_tiles.append(ckv)

                    for h in range(HPC):
                        ps_qc = ps_u.tile([128, 512], F32, tag="u", bufs=2, name=f"psqc{b}{tbl}{h}")
                        for c in range(CQT):
                            nc.tensor.matmul(
                                ps_qc[:],
                                r(wuq_sb[:, c * 256 + h * 128 : c * 256 + (h + 1) * 128]),
                                r(cq_tiles[c][:]),
                                start=(c == 0),
                                stop=(c == CQT - 1),
                            )
                        nc.vector.tensor_copy(qc_u[h][:, col], ps_qc[:])
                        ps_kc = ps_u.tile([128, 512], F32, tag="u", bufs=2, name=f"pskc{b}{tbl}{h}")
                        for c in range(CKVT):
                            nc.tensor.matmul(
                                ps_kc[:],
                                r(wuk_sb[:, c * 256 + h * 128 : c * 256 + (h + 1) * 128]),
                                r(ckv_tiles[c][:]),
                                start=(c == 0),
                                stop=(c == CKVT - 1),
                            )
                        nc.vector.tensor_copy(kc_u[h][:, col], ps_kc[:])
                        ps_v = ps_u.tile([128, 512], F32, tag="u", bufs=2, name=f"psv{b}{tbl}{h}")
                        for c in range(CKVT):
                            nc.tensor.matmul(
                                ps_v[:],
                                r(wuv_sb[:, c * 256 + h * 128 : c * 256 + (h + 1) * 128]),
                                r(ckv_tiles[c][:]),
                                start=(c == 0),
                                stop=(c == CKVT - 1),
                            )
                        v_st = pb_small.tile([128, 512], F32, tag="vst", bufs=2, name=f"vst{b}{tbl}{h}")
                        nc.vector.tensor_copy(v_st[:], ps_v[:])
                        # transpose v to key-major for the PV matmul
                        for j in range(4):
                            ps_t = ps_s.tile([128, 128], F32, tag="s", bufs=2, name=f"pst{b}{tbl}{h}{j}")
                            nc.tensor.transpose(
                                ps_t[:], v_st[:, j * 128 : (j + 1) * 128], id_sb[:]
                            )
                            nc.vector.tensor_copy(
                                vk_u[h][:, tbl * 512 + j * 128 : tbl * 512 + (j + 1) * 128],
                                ps_t[:],
                            )

                    ps_qr = ps_u.tile([128, 512], F32, tag="u", bufs=2, name=f"psqr{b}{tbl}")
                    for c in range(CQT):
                        nc.tensor.matmul(
                            ps_qr[:],
                            r(wqr_sb[:, c * 128 : (c + 1) * 128]),
                            r(cq_tiles[c][:]),
                            start=(c == 0),
                            stop=(c == CQT - 1),
                        )
                    rope(qr_u[:, col], ps_qr, tb, 128)

                # causal attention, k-major softmax
                for h in range(HPC):
                    hr = slice(h * RD, (h + 1) * RD)
                    for qb in range(NBB):
                        qcol = slice(qb * 512, (qb + 1) * 512)
                        kmax = 4 * (qb + 1)
                        ps_ov = ps_o.tile([128, 512], F32, tag="o", bufs=2, name=f"pso{b}{h}{qb}")
                        ps_den = ps_misc.tile([1, 512], F32, tag="den", bufs=1, name=f"psd{b}{h}{qb}")
                        for ki in range(kmax):
                            kcol = slice(ki * 128, (ki + 1) * 128)
                            ps_sc = ps_s.tile([128, 512], F32, tag="s", bufs=2, name=f"pss{b}{h}{qb}{ki}")
                            nc.tensor.matmul(
                                ps_sc[:],
                                r(kc_u[h][:, kcol]),
                                r(qc_u[h][:, qcol]),
                                start=True,
                                stop=False,
                            )
                            nc.tensor.matmul(
                                ps_sc[:],
                                r(kr_sb[hr, b * S + ki * 128 : b * S + (ki + 1) * 128]),
                                r(qr_u[hr, qcol]),
                                start=False,
                                stop=True,
                            )
                            p_sb = pb_small.tile([128, 512], F32R, tag="p", bufs=3, name=f"p{b}{h}{qb}{ki}")
                            nc.scalar.activation(p_sb[:], ps_sc[:], AF.Exp, scale=float(SCALE))
                            if ki >= 4 * qb:
                                o = ki - 4 * qb
                                nc.vector.tensor_mul(
                                    p_sb[:], p_sb[:], mask_sb[:, o * 512 : (o + 1) * 512]
                                )
                            nc.tensor.matmul(
                                ps_ov[:],
                                r(vk_u[h][:, kcol]),
                                r(p_sb[:]),
                                start=(ki == 0),
                                stop=(ki == kmax - 1),
                            )
                            nc.tensor.matmul(
                                ps_den[:],
                                r(ones_k),
                                r(p_sb[:]),
                                start=(ki == 0),
                                stop=(ki == kmax - 1),
                            )
                        recip = pb_small.tile([1, 512], F32R, tag="rc", bufs=2, name=f"rc{b}{h}{qb}")
                        with nc.allow_low_precision(reason="f32r softmax recip"):
                            nc.vector.reciprocal(recip[:], ps_den[:])
                        ps_bc = ps_misc.tile([128, 512], F32, tag="bc", bufs=1, name=f"psb{b}{h}{qb}")
                        nc.tensor.matmul(ps_bc[:], r(ones_m), r(recip[:]), start=True, stop=True)
                        bc_sb = pb_small.tile([128, 512], F32, tag="bc", bufs=2, name=f"bc{b}{h}{qb}")
                        nc.scalar.activation(bc_sb[:], ps_bc[:], AF.Copy)
                        o_sb = pb_small.tile([128, 512], F32R, tag="os", bufs=2, name=f"os{b}{h}{qb}")
                        nc.vector.tensor_mul(o_sb[:], ps_ov[:], bc_sb[:])
                        row = (b * NBB + qb) * (HPC * HD) + h * HD
                        nc.sync.dma_start(a2a_in[row : row + HD, :], o_sb[:])

        nc.gpsimd.collective_compute(
            "AllToAll",
            mybir.AluOpType.bypass,
            replica_groups=rg,
            ins=[a2a_in.ap().opt()],
            outs=[a2a_out.ap().opt()],
        )

        # ================= Phase C: out-projection on own 512 tokens ========
        with (
            tc.tile_pool(name="pc_o", bufs=1) as pc_o,
            tc.tile_pool(name="pc_w", bufs=3) as pc_w,
            tc.tile_pool(name="pc_s", bufs=3) as pc_s,
            tc.tile_pool(name="pc_ps", bufs=2, space="PSUM") as pc_ps,
        ):
            of_sb = pc_o.tile([128, ET * 512], F32R)
            for d in range(ET):
                nc.sync.dma_start(
                    of_sb[:, d * 512 : (d + 1) * 512],
                    a2a_out[d * 128 : (d + 1) * 128, :],
                )
            for ec in range(ET):
                wo_sb = pc_w.tile([128, ET * 128], F32R, tag="wo", bufs=3, name=f"wo{ec}")
                for d in range(ET):
                    nc.sync.dma_start(
                        wo_sb[:, d * 128 : (d + 1) * 128],
                        wout_t[d * 128 : (d + 1) * 128, ec * 128 : (ec + 1) * 128],
                    )
                ps = pc_ps.tile([128, 512], F32, tag="pc", bufs=2, name=f"psc{ec}")
                for d in range(ET):
                    nc.tensor.matmul(
                        ps[:],
                        r(wo_sb[:, d * 128 : (d + 1) * 128]),
                        r(of_sb[:, d * 512 : (d + 1) * 512]),
                        start=(d == 0),
                        stop=(d == ET - 1),
                    )
                o_sb = pc_s.tile([128, 512], F32, tag="oc", bufs=3, name=f"oc{ec}")
                nc.vector.tensor_copy(o_sb[:], ps[:])
                nc.sync.dma_start(out_t[ec * 128 : (ec + 1) * 128, :], o_sb[:])

    nc.compile()
    return nc


_NC_CACHE = None


def _get_program():
    global _NC_CACHE
    if _NC_CACHE is None:
        _NC_CACHE = build_program()
    return _NC_CACHE


def _host_tables():
    pos = np.arange(S, dtype=np.float32)
    inv_freq = 1.0 / (10000.0 ** (np.arange(0, RD, 2, dtype=np.float32) / RD))
    freqs = pos[:, None] * inv_freq[None, :]          # [S, 32]
    cos64 = np.concatenate([np.cos(freqs)] * 2, axis=1).T.astype(np.float32)  # [64, S]
    sin64 = np.sin(freqs).T.astype(np.float32)        # [32, S]
    sin_signed = np.concatenate([-sin64, sin64], axis=0)  # [64, S]
    cos_full = np.tile(cos64, (2, 2))                 # [128, T]
    sin_full = np.tile(sin_signed, (2, 2))            # [128, T]
    kk = np.arange(128)[:, None]
    qq = np.arange(512)[None, :]
    mask = np.concatenate(
        [(kk + o * 128 <= qq).astype(np.float32) for o in range(4)], axis=1
    )  # [128, 2048]
    return cos_full, sin_full, mask


def kernel(x, w_dq, w_uq, w_dkv, w_uk, w_uv, w_qr, w_kr, w_out):
    x = np.asarray(x, dtype=np.float32)
    w_dq = np.asarray(w_dq, dtype=np.float32)
    w_uq = np.asarray(w_uq, dtype=np.float32)
    w_dkv = np.asarray(w_dkv, dtype=np.float32)
    w_uk = np.asarray(w_uk, dtype=np.float32)
    w_uv = np.asarray(w_uv, dtype=np.float32)
    w_qr = np.asarray(w_qr, dtype=np.float32)
    w_kr = np.asarray(w_kr, dtype=np.float32)
    w_out = np.asarray(w_out, dtype=np.float32)

    nc = _get_program()
    cos_full, sin_full, mask = _host_tables()

    xt = np.ascontiguousarray(x.reshape(T, E).T)          # [E, T]
    wdq_t = np.ascontiguousarray(w_dq.T)
    wdkv_t = np.ascontiguousarray(w_dkv.T)
    wout_t = np.ascontiguousarray(w_out.T)
    ident = np.eye(128, dtype=np.float32)

    in_maps = []
    for i in range(NC):
        hp = slice(i * HPC * HD, (i + 1) * HPC * HD)      # this core's head dims
        hr = slice(i * HPC * RD, (i + 1) * HPC * RD)      # this core's rope dims
        in_maps.append(
            {
                "xt_loc": np.ascontiguousarray(xt[:, i * TPC : (i + 1) * TPC]),
                "xt_full": xt,
                "wdq_t": wdq_t,
                "wdkv_t": wdkv_t,
                "wkr_t": np.ascontiguousarray(w_kr[hr, :].T),
                "wuq_t": np.ascontiguousarray(w_uq[hp, :].T),
                "wqr_t": np.ascontiguousarray(w_qr[hr, :].T),
                "wuk_t": np.ascontiguousarray(w_uk[hp, :].T),
                "wuv_t": np.ascontiguousarray(w_uv[hp, :].T),
                "wout_t": wout_t,
                "cos_t": cos_full,
                "sin_t": sin_full,
                "mask_t": mask,
                "ones_t": np.ones((128, 128), dtype=np.float32),
                "ident": ident,
            }
        )

    res = bass_utils.run_bass_kernel_spmd(nc, in_maps, core_ids=list(range(NC)))
    out = np.concatenate(
        [np.ascontiguousarray(res.results[i]["out_t"].T) for i in range(NC)], axis=0
    )
    return out.reshape(B, S, E)


def run_profiled(inputs):
    """Used by test.py: run once with NTFF tracing, return (output, exec_time_ns)."""
    sys.path.insert(0, "/root/.axon_site")
    from trn_agent_boot.trn_boot import _ntff_profile_via_ctypes

    hooks_mod = types.ModuleType("antenv.axon_hooks")
    hook = _ntff_profile_via_ctypes("/opt/axon/libaxon_pjrt.so")
    hooks_mod.get_axon_ntff_profile_hook = lambda: hook
    sys.modules["antenv.axon_hooks"] = hooks_mod

    orig = bass_utils.run_bass_kernel_spmd
    holder = {}

    def wrapper(nc, in_maps, core_ids, **kw):
        kw["trace"] = True
        res = orig(nc, in_maps, core_ids, **kw)
        holder["exec_time_ns"] = res.exec_time_ns
        return res

    bass_utils.run_bass_kernel_spmd = wrapper
    try:
        out = kernel(**inputs)
    finally:
        bass_utils.run_bass_kernel_spmd = orig
    return out, holder.get("exec_time_ns")
